# revision 26
# baseline (speedup 1.0000x reference)
"""Trainium2 Bass kernel v3: multi-head attention (B=4, N=2048, C=1024, H=16).

Sharding: 8 cores = 4 batches x 2 head-halves (tensor parallel over heads).
Each core computes q/k/v for its 8 heads over the full 2048 rows of its
batch, attention for those heads, and a per-head-half partial output
projection y_part = O_norm @ Wout[head-slice]; the host sums the two
partials per batch (plus bout). No K/V duplication, no collectives.

v3 exploits PE array tiling concurrency (HW-verified: tile-position
matmuls in disjoint row/col groups stream simultaneously):
- S stays as 64-row-tiled head pairs (concurrent, 512 cyc per key tile).
- AV runs as column-paired M=64 matmuls: both heads' P@V stream at once
  (tile_position (0,0)/(0,64) into one PSUM bank: h0 -> partitions 0-63,
  h1 -> 64-127), halving AV wall time vs the v2 serial form.
- Softmax denominators no longer ride AV as a ones-column (that forced
  M=65, blocking column pairing). Instead a 4x column-tiled quad of M=1
  ones-matmuls (h x par at col positions 0/32/64/96) accumulates all
  rowsums into one PSUM bank at unit end - 512 cyc per key-tile-pair.
- Projections (Q/K/V) and the output projection Y also emit as column
  pairs (M=64, same (128,64) array mode as AV) so the array mode only
  toggles S-block <-> everything-else once per key-tile pair.

exp tiles [128,1024] alternate between the Scalar engine (native Exp,
bias -ln8 folded) and the Vector engine (Schraudolph fast-exp: int16 =
S*128/ln2 + B bitcast as bf16, 7/16 tiles) so both engines share the
softmax load. PSUM egress is ACT/DVE only; GpSimd handles the SBUF-side
normalization multiplies and half the xT input DMAs (weights stream on
the ACT ring, pair-0 column slices first so unit 0 never waits on
full-width chunks). PSUM: 4 banks S double-buffer + 1 bank AV + 3 banks
proj/Y rotation; the rowsum accumulator borrows a proj/Y bank only for
the unit-end block. post(u-1) is hoisted ahead of each unit's first S
so the AV bank clears early. Y of the last three units is deferred into
the tail; units 14/15 normalize via a fast path (reciprocal of the PSUM
rowsums, partition-broadcast via K=1 matmuls) instead of the DMA round
trip.
"""

import numpy as np
import ml_dtypes

B, N, C, H = 4, 2048, 1024, 16
DH = C // H                      # 64
SCALE = DH ** -0.5
NCORES = 8
HC = 8                           # heads per core
PR = 4                           # head pairs per core
CT = C // 128                    # 8 contraction tiles
MT = N // 128                    # 16 key tiles
MPAIR = MT // 2                  # 8 key-tile pairs
NQC = 4                          # query chunks of 512
NU = PR * NQC                    # 16 attention units

LNK = float(np.log(8.0))         # fold exp(S - ln 8) so P' = exp(S)/8
# DVE fast-exp (Schraudolph in bf16 bit-space, rint convert, HW verified):
# int16 = S*(128/ln2) + (127*128 - 3*128 + delta); bitcast as bf16.
A_SCH = 128.0 / np.log(2.0)
B_SCH = 15864.55                 # rint calibration, min weighted softmax err

# which mt exps go to DVE Schraudolph (rest: ACT native exp). 7/16 on
# DVE matches the engines' 0.96 vs 1.2 GHz rates with their copy load.
DVE_EXP_MTS = (1, 3, 5, 7, 9, 11, 13)

_BF16 = ml_dtypes.bfloat16
_cache = {}


def _patch_tile_drain():
    """Walrus in this env rejects >1 sem wait per instruction; split the tail
    Drain's waits into standalone single-wait nops on SP."""
    import concourse.tile as tile
    import concourse.mybir as mybir
    from concourse.vector_clock import ScopedClock

    if getattr(tile.TileContext, "_drain_split_patched", False):
        return

    def _patched(self, tick_clock, wait_clock):
        nc = self.nc
        drain_inst = nc.sync.drain()
        wait_clock.add_sem_waits(
            drain_inst.ins, ScopedClock({None: tick_clock.global_clock})
        )
        si = drain_inst.ins.sync_info
        waits = list(si.on_wait) if si is not None and si.on_wait else []
        if len(waits) > 1:
            si.on_wait = []
            for w in waits:
                nop = nc.sync.nop(hint="drain_wait_split", nofuse=True)
                nsi = nop.ins.sync_info
                if nsi is None:
                    nop.ins.sync_info = mybir.SyncInfo(on_wait=[w], on_update=[])
                else:
                    nsi.on_wait = [w]
        nc.all_engine_barrier()
        assert self.sems is not None
        popped = nc._tile_sem_poison_stack.pop()
        assert popped is self._sem_poison
        nc.clear_and_free_semaphores(list(self.sems.allocated().values()))
        nc.all_engine_barrier()

    tile.TileContext._drain_and_barrier = _patched
    tile.TileContext._drain_split_patched = True


def _split_excess_waits(nc, limit=1):
    """Walrus here rejects instructions carrying more than `limit` sem waits.
    Move the excess onto same-engine nops inserted immediately before."""
    import concourse.mybir as mybir

    counter = [0]
    for block in nc.m.functions[0].blocks:
        il = block.instructions
        i = 0
        while i < len(il):
            inst = il[i]
            si = inst.sync_info
            waits = list(si.on_wait) if si is not None and si.on_wait else []
            if len(waits) > limit:
                keep = waits[-limit:]
                extra = waits[:-limit]
                si.on_wait = keep
                pos = i
                for j in range(0, len(extra), limit):
                    chunk = extra[j : j + limit]
                    counter[0] += 1
                    nop = mybir.InstNoOp(
                        name=f"waitsplit_{counter[0]}",
                        engine=inst.engine,
                        ins=[],
                        outs=[],
                        sync_info=mybir.SyncInfo(on_wait=chunk, on_update=[]),
                    )
                    try:
                        nc.register_instruction(nop, overwrite=True)
                    except Exception:
                        pass
                    il.insert(pos, nop)
                    pos += 1
                    i += 1
            i += 1


def build_nc():
    import concourse.bass as bass
    import concourse.mybir as mybir
    import concourse.tile as tile

    _patch_tile_drain()
    f32 = mybir.dt.float32
    bf16 = mybir.dt.bfloat16
    i16 = mybir.dt.int16
    EXP = mybir.ActivationFunctionType.Exp
    MUL = mybir.AluOpType.mult
    ADD = mybir.AluOpType.add

    nc = bass.Bass("TRN2", num_devices=NCORES)
    xT = nc.dram_tensor("xT", [C, N], bf16, kind="ExternalInput")
    Wq = nc.dram_tensor("Wq", [C, 512], bf16, kind="ExternalInput")
    Wk = nc.dram_tensor("Wk", [C, 512], bf16, kind="ExternalInput")
    Wv = nc.dram_tensor("Wv", [C, 512], bf16, kind="ExternalInput")
    Wout = nc.dram_tensor("Wout", [512, C], bf16, kind="ExternalInput")
    yp = nc.dram_tensor("yp", [PR, N, C], bf16, kind="ExternalOutput")

    xT_r = xT.ap().rearrange("(a p) n -> p a n", p=128)
    Wq_r = Wq.ap().rearrange("(a p) d -> p a d", p=128)
    Wk_r = Wk.ap().rearrange("(a p) d -> p a d", p=128)
    Wv_r = Wv.ap().rearrange("(a p) d -> p a d", p=128)
    Wout_r = Wout.ap().rearrange("(a p) d -> p a d", p=128)

    with tile.TileContext(nc) as tc:
      with (
          tc.tile_pool(name="persist", bufs=1) as persist,
          tc.tile_pool(name="small", bufs=2) as small,
          tc.tile_pool(name="ys_pool", bufs=4) as ys_pool,
          tc.tile_pool(name="dramp", bufs=1, space="DRAM") as dram_pool,
          tc.tile_pool(name="ps_st", bufs=2, space="PSUM") as ps_st,
          tc.tile_pool(name="ps_ot", bufs=1, space="PSUM") as ps_ot,
          tc.tile_pool(name="ps_pp", bufs=3, space="PSUM") as ps_pp,
      ):
        xT_t = persist.tile([128, CT, N], bf16, name="xT_t")
        Wq_t = persist.tile([128, CT, 512], bf16, name="Wq_t")
        Wk_t = persist.tile([128, CT, 512], bf16, name="Wk_t")
        Wv_t = persist.tile([128, CT, 512], bf16, name="Wv_t")
        Wout_t = persist.tile([128, PR, C], bf16, name="Wout_t")
        qT_sb = [persist.tile([128, N], bf16, name=f"qT{p}") for p in range(PR)]
        kT_sb = [persist.tile([128, N], bf16, name=f"kT{p}") for p in range(PR)]
        v8 = persist.tile([128, MT, HC, DH], bf16, name="v8")
        pt = persist.tile([128, MPAIR, 2, 1024], bf16, name="pt")
        OT = [persist.tile([128, N], bf16, name=f"OT{p}") for p in range(PR)]
        bias_exp = persist.tile([128, 1], f32, name="bias_exp")
        ones_sb = persist.tile([128, 64], bf16, name="ones_sb")
        onesf_sb = persist.tile([128, 64], f32, name="onesf_sb")
        rinv_dram = dram_pool.tile([HC, N], f32)
        rs_dram = dram_pool.tile([HC, N], f32)

        # ---- DMA kickoff (chunked so compute starts early); xT owns the
        # sync+gpsimd rings, weights go out on the other engines' rings ----
        for jc in range(CT):
            eng = nc.sync if jc % 2 == 0 else nc.gpsimd
            eng.dma_start(out=xT_t[:, jc, :], in_=xT_r[:, jc, :])
            # pair-0 column slices first: unit 0's q/k projections only
            # read cols 0:128, so they never wait on the full-width chunks
            nc.scalar.dma_start(out=Wq_t[:, jc, 0:128], in_=Wq_r[:, jc, 0:128])
            nc.scalar.dma_start(out=Wk_t[:, jc, 0:128], in_=Wk_r[:, jc, 0:128])
        for jc in range(CT):
            nc.scalar.dma_start(out=Wq_t[:, jc, 128:512], in_=Wq_r[:, jc, 128:512])
            nc.scalar.dma_start(out=Wk_t[:, jc, 128:512], in_=Wk_r[:, jc, 128:512])
        for jc in range(CT):
            nc.scalar.dma_start(out=Wv_t[:, jc, :], in_=Wv_r[:, jc, :])
        nc.scalar.dma_start(out=Wout_t, in_=Wout_r)
        nc.vector.memset(bias_exp, -LNK)
        rs2 = persist.tile([128, 2048], f32, name="rs2")
        rinv2 = persist.tile([128, 2048], f32, name="rinv2")
        nc.vector.memset(rs2, 1.0)
        nc.vector.memset(ones_sb, 1.0)
        nc.vector.memset(onesf_sb, 1.0)

        # ---- projection emitters (column-paired: M=64, (128,64) mode) ----
        def emit_q(p, ch):
            ps = ps_pp.tile([128, 512], f32, tag="pp", name=f"psq{p}_{ch}")
            for jc in range(CT):
                for half in range(2):
                    nc.tensor.matmul(
                        ps[half * 64 : (half + 1) * 64, :],
                        Wq_t[:, jc, p * 128 + half * 64 : p * 128 + (half + 1) * 64],
                        xT_t[:, jc, ch * 512 : (ch + 1) * 512],
                        start=(jc == 0), stop=(jc == CT - 1),
                        tile_position=(0, half * 64),
                    )
            dst = qT_sb[p][:, ch * 512 : (ch + 1) * 512]
            if (p + ch) % 2:
                nc.vector.tensor_copy(out=dst, in_=ps)
            else:
                nc.scalar.copy(out=dst, in_=ps)

        def emit_k(p, ch):
            ps = ps_pp.tile([128, 512], f32, tag="pp", name=f"psk{p}_{ch}")
            for jc in range(CT):
                for half in range(2):
                    nc.tensor.matmul(
                        ps[half * 64 : (half + 1) * 64, :],
                        Wk_t[:, jc, p * 128 + half * 64 : p * 128 + (half + 1) * 64],
                        xT_t[:, jc, ch * 512 : (ch + 1) * 512],
                        start=(jc == 0), stop=(jc == CT - 1),
                        tile_position=(0, half * 64),
                    )
            dst = kT_sb[p][:, ch * 512 : (ch + 1) * 512]
            if (p + ch) % 2:
                nc.scalar.copy(out=dst, in_=ps)
            else:
                nc.vector.tensor_copy(out=dst, in_=ps)

        def emit_v(mt):
            ps = ps_pp.tile([128, 512], f32, tag="pp", name=f"psv{mt}")
            for jc in range(CT):
                for half in range(2):
                    nc.tensor.matmul(
                        ps[half * 64 : (half + 1) * 64, :],
                        xT_t[:, jc, mt * 128 + half * 64 : mt * 128 + (half + 1) * 64],
                        Wv_t[:, jc, :],
                        start=(jc == 0), stop=(jc == CT - 1),
                        tile_position=(0, half * 64),
                    )
            nc.vector.tensor_copy(
                out=v8[:, mt, :, :], in_=ps.rearrange("p (h d) -> p h d", h=HC)
            )

        # ---- attention unit pieces ----
        def emit_s(u, mt):
            p, qc = u // NQC, u % NQC
            ms = slice(mt * 128, (mt + 1) * 128)
            qs = slice(qc * 512, (qc + 1) * 512)
            st = ps_st.tile([128, 1024], f32, tag="st", name=f"st{u}_{mt}")
            nc.tensor.matmul(
                st[:, 0:512], kT_sb[p][0:64, ms], qT_sb[p][0:64, qs],
                start=True, stop=True, tile_position=(0, 0),
            )
            nc.tensor.matmul(
                st[:, 512:1024], kT_sb[p][64:128, ms], qT_sb[p][64:128, qs],
                start=True, stop=True, tile_position=(64, 0),
            )
            return st

        def emit_exp(u, mt, st):
            dst = pt[:, mt // 2, mt % 2, :]
            if mt in DVE_EXP_MTS:
                nc.vector.tensor_scalar(
                    out=dst.bitcast(i16), in0=st,
                    scalar1=A_SCH, scalar2=B_SCH, op0=MUL, op1=ADD,
                )
            else:
                nc.scalar.activation(out=dst, in_=st, func=EXP, bias=bias_exp)

        def emit_av(u, mp, ot_t):
            p = u // NQC
            for par in range(2):
                for h in range(2):
                    nc.tensor.matmul(
                        ot_t[h * 64 : (h + 1) * 64, :],
                        v8[:, 2 * mp + par, 2 * p + h, :],
                        pt[:, mp, par, h * 512 : (h + 1) * 512],
                        start=(mp == 0 and par == 0),
                        stop=(mp == MPAIR - 1 and par == 1),
                        tile_position=(0, h * 64),
                    )

        def emit_rs(u, mp, rs_t):
            # 4x col-tiled M=1 ones-matmuls: rowsums for (h, q-quarter) at
            # psum partitions 0/32/64/96, accumulated over mp AND par so no
            # cross-partition combine is needed afterwards. F=256 each; the
            # four tiles stream concurrently (256 cyc per (mp, par) slot).
            for par in range(2):
                for quad in range(4):
                    pp = 32 * quad
                    nc.tensor.matmul(
                        rs_t[pp : pp + 1, 0:256],
                        ones_sb[:, 0:1],
                        pt[:, mp, par, quad * 256 : (quad + 1) * 256],
                        start=(mp == 0 and par == 0),
                        stop=(mp == MPAIR - 1 and par == 1),
                        tile_position=(0, pp),
                    )

        # trailing work for a finished unit u: copy O, rowsums -> rinv
        def emit_post(u, ot_t, rs_t, fast):
            p, qc = u // NQC, u % NQC
            qs = slice(qc * 512, (qc + 1) * 512)
            nc.scalar.copy(out=OT[p][:, qs], in_=ot_t)
            if fast:
                # rowsums straight into rs2 rows (h*64): quarters live at
                # psum partitions 32*(2h+g)
                off = 512 * (u % 4)
                for h in range(2):
                    for g in range(2):
                        src = rs_t[32 * (2 * h + g) : 32 * (2 * h + g) + 1, 0:256]
                        nc.vector.tensor_copy(
                            out=rs2[
                                h * 64 : h * 64 + 1,
                                off + g * 256 : off + (g + 1) * 256,
                            ],
                            in_=src,
                        )
                return
            for h in range(2):
                rs_row = small.tile(
                    [1, 512], f32, tag=f"rsrow{h}", name=f"rsr{u}_{h}"
                )
                for g in range(2):
                    nc.vector.tensor_copy(
                        out=rs_row[:, g * 256 : (g + 1) * 256],
                        in_=rs_t[32 * (2 * h + g) : 32 * (2 * h + g) + 1, 0:256],
                    )
                nc.sync.dma_start(
                    out=rs_dram[2 * p + h : 2 * p + h + 1, qs], in_=rs_row
                )
            rs_g = small.tile([128, 2, 4], f32, tag="rs", name=f"rs{u}")
            nc.sync.dma_start(
                out=rs_g,
                in_=rs_dram[2 * p : 2 * p + 2, qs].rearrange(
                    "h (p f) -> p h f", f=4
                ),
            )
            rinv_t = small.tile([128, 2, 4], f32, tag="ri", name=f"ri{u}")
            nc.vector.reciprocal(out=rinv_t, in_=rs_g)
            nc.sync.dma_start(
                out=rinv_dram[2 * p : 2 * p + 2, qs].rearrange(
                    "h (p f) -> p h f", f=4
                ),
                in_=rinv_t,
            )

        def emit_norm(u):
            p, qc = u // NQC, u % NQC
            qs = slice(qc * 512, (qc + 1) * 512)
            rbc = small.tile([128, 512], f32, tag="rbc", name=f"rbc{u}")
            for h in range(2):
                nc.sync.dma_start(
                    out=rbc[h * 64 : (h + 1) * 64, :],
                    in_=rinv_dram[
                        2 * p + h : 2 * p + h + 1, qs
                    ].to_broadcast([64, 512]),
                )
            nc.gpsimd.tensor_mul(OT[p][:, qs], OT[p][:, qs], rbc)

        def emit_norm_fast(u):
            # reciprocal off the PSUM rowsums already in rs2; partition-
            # broadcast via K=1 matmuls; multiply on DVE.
            p, qc = u // NQC, u % NQC
            qs = slice(qc * 512, (qc + 1) * 512)
            off = 512 * (u % 4)
            nc.vector.reciprocal(
                out=rinv2[:, off : off + 512], in_=rs2[:, off : off + 512]
            )
            rbc = ps_pp.tile([128, 512], f32, tag="pp", name=f"rbcf{u}")
            for h in range(2):
                nc.tensor.matmul(
                    rbc[h * 64 : (h + 1) * 64, :],
                    onesf_sb[h * 64 : h * 64 + 1, :],
                    rinv2[h * 64 : h * 64 + 1, off : off + 512],
                    start=True, stop=True,
                )
            nc.vector.tensor_mul(OT[p][:, qs], OT[p][:, qs], rbc)

        def emit_y(u, t):
            p, qc = u // NQC, u % NQC
            qt = qc * 4 + t
            ys = ys_pool.tile([128, C], bf16, tag="ys", name=f"ys{u}_{t}")
            for ch in range(2):
                ps = ps_pp.tile([128, 512], f32, tag="pp", name=f"psy{u}{t}{ch}")
                for half in range(2):
                    nc.tensor.matmul(
                        ps[half * 64 : (half + 1) * 64, :],
                        OT[p][:, qt * 128 + half * 64 : qt * 128 + (half + 1) * 64],
                        Wout_t[:, p, ch * 512 : (ch + 1) * 512],
                        start=True, stop=True,
                        tile_position=(0, half * 64),
                    )
                if ch == 0:
                    nc.scalar.copy(out=ys[:, 0:512], in_=ps)
                else:
                    nc.vector.tensor_copy(out=ys[:, 512:1024], in_=ps)
            nc.sync.dma_start(
                out=yp.ap()[p, qt * 128 : (qt + 1) * 128, :], in_=ys
            )

        # ---- weave schedules ----
        proj_sched = {
            0: [lambda: emit_q(0, 1), lambda: emit_k(0, 1),
                lambda: emit_k(0, 2), lambda: emit_k(0, 3)]
               + [lambda mt=mt: emit_v(mt) for mt in range(MT)],
            1: [lambda: emit_q(0, 2), lambda: emit_k(1, 0), lambda: emit_k(1, 1)],
            2: [lambda: emit_q(0, 3), lambda: emit_k(1, 2), lambda: emit_q(1, 0)],
            3: [lambda: emit_k(1, 3), lambda: emit_q(1, 1)],
            4: [lambda: emit_q(1, 2), lambda: emit_k(2, 0)],
            5: [lambda: emit_q(1, 3), lambda: emit_k(2, 1)],
            6: [lambda: emit_k(2, 2), lambda: emit_q(2, 0)],
            7: [lambda: emit_k(2, 3), lambda: emit_q(2, 1)],
            8: [lambda: emit_q(2, 2), lambda: emit_k(3, 0)],
            9: [lambda: emit_q(2, 3), lambda: emit_k(3, 1)],
            10: [lambda: emit_k(3, 2), lambda: emit_q(3, 0)],
            11: [lambda: emit_k(3, 3), lambda: emit_q(3, 1)],
            12: [lambda: emit_q(3, 2)],
            13: [lambda: emit_q(3, 3)],
        }

        # ---- bootstrap ----
        emit_q(0, 0)
        emit_k(0, 0)

        prev = None          # (u-1, ot_t, rs_t) for post
        norm_u = None        # unit whose norm should be emitted
        y_u = None           # unit whose Y should be emitted

        for u in range(NU):
            queue = []
            if prev is not None:
                up, pot, prs = prev
                emit_post(up, pot, prs, fast=(up >= NU - 3))
            queue.extend(proj_sched.get(u, []))
            if norm_u is not None and norm_u < NU - 3:
                queue.append(lambda nu=norm_u: emit_norm(nu))
            if y_u is not None and y_u <= NU - 4:
                for t in range(4):
                    queue.append(lambda yu=y_u, t=t: emit_y(yu, t))
            if prev is not None and prev[0] >= NU - 3:
                # fast norm is the LAST pop (mt=15): its DVE inputs (rs2
                # from the hoisted post) are long since ready, so the PE
                # broadcast matmuls never stall the in-order queue
                queue.append(lambda nu=prev[0]: emit_norm_fast(nu))

            ot_t = ps_ot.tile([128, 512], f32, tag="ot", name=f"ot{u}")
            nq = len(queue)
            popped = 0
            if u == 0:
                # AV(mp) legal only after emit_v(2mp+1) popped; queue has
                # 4 q/k emits then 16 V emits: V(j) is queue item 4+j.
                pending_av = list(range(MPAIR))
            for mt in range(MT):
                st = emit_s(u, mt)
                emit_exp(u, mt, st)
                if mt % 2 == 0:
                    continue
                # pop weave items at mt-pair boundaries so the PE array
                # mode only toggles S-block <-> (128,64)-block per pair.
                # Hold back the last few items to fill the unit-end block
                # while exp(14)/exp(15) complete.
                want = max((mt + 1) * nq // MT, 1)
                if u > 0:
                    want = min(want, max(nq - 3, 1))
                while popped < want:
                    queue[popped]()
                    popped += 1
                if u == 0:
                    while pending_av and 4 + 2 * pending_av[0] + 1 < popped:
                        emit_av(u, pending_av.pop(0), ot_t)
                elif mt >= 5:
                    emit_av(u, (mt - 5) // 2, ot_t)
            if u == 0:
                while popped < nq:
                    queue[popped]()
                    popped += 1
                while pending_av:
                    emit_av(u, pending_av.pop(0), ot_t)
            else:
                emit_av(u, MPAIR - 2, ot_t)
                while popped < nq:
                    queue[popped]()
                    popped += 1
                emit_av(u, MPAIR - 1, ot_t)
            # rowsum accumulator borrows a pp bank only for the unit-end
            # rs block (the bank was idle there anyway); readers in
            # post(u) release it early in unit u+1.
            rs_t = ps_pp.tile([128, 512], f32, tag="pp", name=f"rsp{u}")
            for mp in range(MPAIR):
                emit_rs(u, mp, rs_t)

            prev = (u, ot_t, rs_t)
            norm_u = u - 1 if u >= 1 else None
            y_u = u - 2 if u >= 2 else None

        # ---- tail: units 13-15 were fast-normed right after their posts;
        # only unit 15's post/norm plus the deferred Ys remain ----
        up, pot, prs = prev
        emit_post(up, pot, prs, fast=True)
        for t in range(4):
            emit_y(NU - 3, t)
        emit_norm_fast(NU - 1)
        for t in range(4):
            emit_y(NU - 2, t)
        for t in range(4):
            emit_y(NU - 1, t)

    _split_excess_waits(nc)
    return nc


def make_in_maps(x, Wq, Wkv, Wout, bout):
    x = np.asarray(x, dtype=np.float32)
    Wq = np.asarray(Wq, dtype=np.float32)
    Wkv = np.asarray(Wkv, dtype=np.float32)
    Wout = np.asarray(Wout, dtype=np.float32)
    Wq_s = Wq * SCALE
    Wk = Wkv[:, :C]
    Wv = Wkv[:, C:]
    in_maps = []
    for core in range(NCORES):
        b, g = core // 2, core % 2
        cs = slice(g * 512, (g + 1) * 512)
        in_maps.append(
            dict(
                xT=np.ascontiguousarray(x[b].T).astype(_BF16),
                Wq=np.ascontiguousarray(Wq_s[:, cs]).astype(_BF16),
                Wk=np.ascontiguousarray(Wk[:, cs]).astype(_BF16),
                Wv=np.ascontiguousarray(Wv[:, cs]).astype(_BF16),
                Wout=np.ascontiguousarray(Wout[cs, :]).astype(_BF16),
            )
        )
    return in_maps


def assemble(results, bout):
    bout = np.asarray(bout, dtype=np.float32)
    out = np.empty((B, N, C), dtype=np.float32)
    for b in range(B):
        acc = results[2 * b]["yp"].astype(np.float32).sum(axis=0)
        acc += results[2 * b + 1]["yp"].astype(np.float32).sum(axis=0)
        out[b] = acc + bout
    return out


def kernel(x, Wq, Wkv, Wout, bout):
    from concourse.bass_utils import run_bass_kernel_spmd

    if "nc" not in _cache:
        _cache["nc"] = build_nc()
    in_maps = make_in_maps(x, Wq, Wkv, Wout, bout)
    res = run_bass_kernel_spmd(_cache["nc"], in_maps, core_ids=list(range(NCORES)))
    return assemble(res.results, bout)


# revision 28
# speedup vs baseline: 1.0340x; 1.0340x over previous
"""Trainium2 Bass kernel v3: multi-head attention (B=4, N=2048, C=1024, H=16).

Sharding: 8 cores = 4 batches x 2 head-halves (tensor parallel over heads).
Each core computes q/k/v for its 8 heads over the full 2048 rows of its
batch, attention for those heads, and a per-head-half partial output
projection y_part = O_norm @ Wout[head-slice]; the host sums the two
partials per batch (plus bout). No K/V duplication, no collectives.

v3 exploits PE array tiling concurrency (HW-verified: tile-position
matmuls in disjoint row/col groups stream simultaneously):
- S stays as 64-row-tiled head pairs (concurrent, 512 cyc per key tile).
- AV runs as column-paired M=64 matmuls: both heads' P@V stream at once
  (tile_position (0,0)/(0,64) into one PSUM bank: h0 -> partitions 0-63,
  h1 -> 64-127), halving AV wall time vs the v2 serial form.
- Softmax denominators no longer ride AV as a ones-column (that forced
  M=65, blocking column pairing). Instead a 4x column-tiled quad of M=1
  ones-matmuls (h x par at col positions 0/32/64/96) accumulates all
  rowsums into one PSUM bank at unit end - 512 cyc per key-tile-pair.
- Projections (Q/K/V) and the output projection Y also emit as column
  pairs (M=64, same (128,64) array mode as AV) so the array mode only
  toggles S-block <-> everything-else once per key-tile pair.

exp tiles [128,1024] alternate between the Scalar engine (native Exp,
bias -ln8 folded) and the Vector engine (Schraudolph fast-exp: int16 =
S*128/ln2 + B bitcast as bf16, 7/16 tiles) so both engines share the
softmax load. PSUM egress is ACT/DVE only; GpSimd handles the SBUF-side
normalization multiplies and half the xT input DMAs (weights stream on
the ACT ring, pair-0 column slices first so unit 0 never waits on
full-width chunks). PSUM: 4 banks S double-buffer + 1 bank AV + 3 banks
proj/Y rotation; the rowsum accumulator borrows a proj/Y bank only for
the unit-end block. post(u-1) is hoisted ahead of each unit's first S
so the AV bank clears early. Y of the last three units is deferred into
the tail; units 14/15 normalize via a fast path (reciprocal of the PSUM
rowsums, partition-broadcast via K=1 matmuls) instead of the DMA round
trip.
"""

import numpy as np
import ml_dtypes

B, N, C, H = 4, 2048, 1024, 16
DH = C // H                      # 64
SCALE = DH ** -0.5
NCORES = 8
HC = 8                           # heads per core
PR = 4                           # head pairs per core
CT = C // 128                    # 8 contraction tiles
MT = N // 128                    # 16 key tiles
MPAIR = MT // 2                  # 8 key-tile pairs
NQC = 4                          # query chunks of 512
NU = PR * NQC                    # 16 attention units

LNK = float(np.log(8.0))         # fold exp(S - ln 8) so P' = exp(S)/8
# DVE fast-exp (Schraudolph in bf16 bit-space, rint convert, HW verified):
# int16 = S*(128/ln2) + (127*128 - 3*128 + delta); bitcast as bf16.
A_SCH = 128.0 / np.log(2.0)
B_SCH = 15864.55                 # rint calibration, min weighted softmax err

# which mt exps go to DVE Schraudolph (rest: ACT native exp). 7/16 on
# DVE matches the engines' 0.96 vs 1.2 GHz rates with their copy load.
DVE_EXP_MTS = (3, 5, 7, 9, 11, 13, 15)

_BF16 = ml_dtypes.bfloat16
_cache = {}


def _patch_tile_drain():
    """Walrus in this env rejects >1 sem wait per instruction; split the tail
    Drain's waits into standalone single-wait nops on SP."""
    import concourse.tile as tile
    import concourse.mybir as mybir
    from concourse.vector_clock import ScopedClock

    if getattr(tile.TileContext, "_drain_split_patched", False):
        return

    def _patched(self, tick_clock, wait_clock):
        nc = self.nc
        drain_inst = nc.sync.drain()
        wait_clock.add_sem_waits(
            drain_inst.ins, ScopedClock({None: tick_clock.global_clock})
        )
        si = drain_inst.ins.sync_info
        waits = list(si.on_wait) if si is not None and si.on_wait else []
        if len(waits) > 1:
            si.on_wait = []
            for w in waits:
                nop = nc.sync.nop(hint="drain_wait_split", nofuse=True)
                nsi = nop.ins.sync_info
                if nsi is None:
                    nop.ins.sync_info = mybir.SyncInfo(on_wait=[w], on_update=[])
                else:
                    nsi.on_wait = [w]
        nc.all_engine_barrier()
        assert self.sems is not None
        popped = nc._tile_sem_poison_stack.pop()
        assert popped is self._sem_poison
        nc.clear_and_free_semaphores(list(self.sems.allocated().values()))
        nc.all_engine_barrier()

    tile.TileContext._drain_and_barrier = _patched
    tile.TileContext._drain_split_patched = True


def _split_excess_waits(nc, limit=1):
    """Walrus here rejects instructions carrying more than `limit` sem waits.
    Move the excess onto same-engine nops inserted immediately before."""
    import concourse.mybir as mybir

    counter = [0]
    for block in nc.m.functions[0].blocks:
        il = block.instructions
        i = 0
        while i < len(il):
            inst = il[i]
            si = inst.sync_info
            waits = list(si.on_wait) if si is not None and si.on_wait else []
            if len(waits) > limit:
                keep = waits[-limit:]
                extra = waits[:-limit]
                si.on_wait = keep
                pos = i
                for j in range(0, len(extra), limit):
                    chunk = extra[j : j + limit]
                    counter[0] += 1
                    nop = mybir.InstNoOp(
                        name=f"waitsplit_{counter[0]}",
                        engine=inst.engine,
                        ins=[],
                        outs=[],
                        sync_info=mybir.SyncInfo(on_wait=chunk, on_update=[]),
                    )
                    try:
                        nc.register_instruction(nop, overwrite=True)
                    except Exception:
                        pass
                    il.insert(pos, nop)
                    pos += 1
                    i += 1
            i += 1


def build_nc():
    import concourse.bass as bass
    import concourse.mybir as mybir
    import concourse.tile as tile

    _patch_tile_drain()
    f32 = mybir.dt.float32
    bf16 = mybir.dt.bfloat16
    i16 = mybir.dt.int16
    EXP = mybir.ActivationFunctionType.Exp
    MUL = mybir.AluOpType.mult
    ADD = mybir.AluOpType.add

    nc = bass.Bass("TRN2", num_devices=NCORES)
    xT = nc.dram_tensor("xT", [C, N], bf16, kind="ExternalInput")
    Wq = nc.dram_tensor("Wq", [C, 512], bf16, kind="ExternalInput")
    Wk = nc.dram_tensor("Wk", [C, 512], bf16, kind="ExternalInput")
    Wv = nc.dram_tensor("Wv", [C, 512], bf16, kind="ExternalInput")
    Wout = nc.dram_tensor("Wout", [512, C], bf16, kind="ExternalInput")
    yp = nc.dram_tensor("yp", [PR, N, C], bf16, kind="ExternalOutput")

    xT_r = xT.ap().rearrange("(a p) n -> p a n", p=128)
    Wq_r = Wq.ap().rearrange("(a p) d -> p a d", p=128)
    Wk_r = Wk.ap().rearrange("(a p) d -> p a d", p=128)
    Wv_r = Wv.ap().rearrange("(a p) d -> p a d", p=128)
    Wout_r = Wout.ap().rearrange("(a p) d -> p a d", p=128)

    with tile.TileContext(nc) as tc:
      with (
          tc.tile_pool(name="persist", bufs=1) as persist,
          tc.tile_pool(name="small", bufs=2) as small,
          tc.tile_pool(name="ys_pool", bufs=4) as ys_pool,
          tc.tile_pool(name="dramp", bufs=1, space="DRAM") as dram_pool,
          tc.tile_pool(name="ps_st", bufs=2, space="PSUM") as ps_st,
          tc.tile_pool(name="ps_ot", bufs=1, space="PSUM") as ps_ot,
          tc.tile_pool(name="ps_pp", bufs=3, space="PSUM") as ps_pp,
      ):
        xT_t = persist.tile([128, CT, N], bf16, name="xT_t")
        Wq_t = persist.tile([128, CT, 512], bf16, name="Wq_t")
        Wk_t = persist.tile([128, CT, 512], bf16, name="Wk_t")
        Wv_t = persist.tile([128, CT, 512], bf16, name="Wv_t")
        Wout_t = persist.tile([128, PR, C], bf16, name="Wout_t")
        qT_sb = [persist.tile([128, N], bf16, name=f"qT{p}") for p in range(PR)]
        kT_sb = [persist.tile([128, N], bf16, name=f"kT{p}") for p in range(PR)]
        v8 = persist.tile([128, MT, HC, DH], bf16, name="v8")
        pt = persist.tile([128, MPAIR, 2, 1024], bf16, name="pt")
        OT = [persist.tile([128, N], bf16, name=f"OT{p}") for p in range(PR)]
        bias_exp = persist.tile([128, 1], f32, name="bias_exp")
        ones_sb = persist.tile([128, 64], bf16, name="ones_sb")
        onesf_sb = persist.tile([128, 64], f32, name="onesf_sb")
        rinv_dram = dram_pool.tile([HC, N], f32)
        rs_dram = dram_pool.tile([HC, N], f32)

        # ---- DMA kickoff (chunked so compute starts early); xT owns the
        # sync+gpsimd rings, weights go out on the other engines' rings ----
        for jc in range(CT):
            eng = nc.sync if jc % 2 == 0 else nc.gpsimd
            eng.dma_start(out=xT_t[:, jc, :], in_=xT_r[:, jc, :])
            # pair-0 column slices first: unit 0's q/k projections only
            # read cols 0:128, so they never wait on the full-width chunks
            nc.scalar.dma_start(out=Wq_t[:, jc, 0:128], in_=Wq_r[:, jc, 0:128])
            nc.scalar.dma_start(out=Wk_t[:, jc, 0:128], in_=Wk_r[:, jc, 0:128])
        for jc in range(CT):
            nc.scalar.dma_start(out=Wq_t[:, jc, 128:512], in_=Wq_r[:, jc, 128:512])
            nc.scalar.dma_start(out=Wk_t[:, jc, 128:512], in_=Wk_r[:, jc, 128:512])
        for jc in range(CT):
            nc.scalar.dma_start(out=Wv_t[:, jc, :], in_=Wv_r[:, jc, :])
        nc.scalar.dma_start(out=Wout_t, in_=Wout_r)
        nc.vector.memset(bias_exp, -LNK)
        rs2 = persist.tile([128, 2048], f32, name="rs2")
        rinv2 = persist.tile([128, 2048], f32, name="rinv2")
        nc.vector.memset(rs2, 1.0)
        nc.vector.memset(ones_sb, 1.0)
        nc.vector.memset(onesf_sb, 1.0)

        # ---- projection emitters (column-paired: M=64, (128,64) mode) ----
        def emit_q(p, ch):
            ps = ps_pp.tile([128, 512], f32, tag="pp", name=f"psq{p}_{ch}")
            for jc in range(CT):
                for half in range(2):
                    nc.tensor.matmul(
                        ps[half * 64 : (half + 1) * 64, :],
                        Wq_t[:, jc, p * 128 + half * 64 : p * 128 + (half + 1) * 64],
                        xT_t[:, jc, ch * 512 : (ch + 1) * 512],
                        start=(jc == 0), stop=(jc == CT - 1),
                        tile_position=(0, half * 64),
                    )
            dst = qT_sb[p][:, ch * 512 : (ch + 1) * 512]
            if (p + ch) % 2:
                nc.vector.tensor_copy(out=dst, in_=ps)
            else:
                nc.scalar.copy(out=dst, in_=ps)

        def emit_k(p, ch):
            ps = ps_pp.tile([128, 512], f32, tag="pp", name=f"psk{p}_{ch}")
            for jc in range(CT):
                for half in range(2):
                    nc.tensor.matmul(
                        ps[half * 64 : (half + 1) * 64, :],
                        Wk_t[:, jc, p * 128 + half * 64 : p * 128 + (half + 1) * 64],
                        xT_t[:, jc, ch * 512 : (ch + 1) * 512],
                        start=(jc == 0), stop=(jc == CT - 1),
                        tile_position=(0, half * 64),
                    )
            dst = kT_sb[p][:, ch * 512 : (ch + 1) * 512]
            if (p + ch) % 2:
                nc.scalar.copy(out=dst, in_=ps)
            else:
                nc.vector.tensor_copy(out=dst, in_=ps)

        def emit_v(mt):
            ps = ps_pp.tile([128, 512], f32, tag="pp", name=f"psv{mt}")
            for jc in range(CT):
                for half in range(2):
                    nc.tensor.matmul(
                        ps[half * 64 : (half + 1) * 64, :],
                        xT_t[:, jc, mt * 128 + half * 64 : mt * 128 + (half + 1) * 64],
                        Wv_t[:, jc, :],
                        start=(jc == 0), stop=(jc == CT - 1),
                        tile_position=(0, half * 64),
                    )
            nc.vector.tensor_copy(
                out=v8[:, mt, :, :], in_=ps.rearrange("p (h d) -> p h d", h=HC)
            )

        # ---- attention unit pieces ----
        def emit_s(u, mt):
            p, qc = u // NQC, u % NQC
            ms = slice(mt * 128, (mt + 1) * 128)
            qs = slice(qc * 512, (qc + 1) * 512)
            st = ps_st.tile([128, 1024], f32, tag="st", name=f"st{u}_{mt}")
            nc.tensor.matmul(
                st[:, 0:512], kT_sb[p][0:64, ms], qT_sb[p][0:64, qs],
                start=True, stop=True, tile_position=(0, 0),
            )
            nc.tensor.matmul(
                st[:, 512:1024], kT_sb[p][64:128, ms], qT_sb[p][64:128, qs],
                start=True, stop=True, tile_position=(64, 0),
            )
            return st

        def emit_exp(u, mt, st):
            dst = pt[:, mt // 2, mt % 2, :]
            if mt in DVE_EXP_MTS:
                nc.vector.tensor_scalar(
                    out=dst.bitcast(i16), in0=st,
                    scalar1=A_SCH, scalar2=B_SCH, op0=MUL, op1=ADD,
                )
            else:
                nc.scalar.activation(out=dst, in_=st, func=EXP, bias=bias_exp)

        def emit_av(u, mp, ot_t):
            p = u // NQC
            for par in range(2):
                for h in range(2):
                    nc.tensor.matmul(
                        ot_t[h * 64 : (h + 1) * 64, :],
                        v8[:, 2 * mp + par, 2 * p + h, :],
                        pt[:, mp, par, h * 512 : (h + 1) * 512],
                        start=(mp == 0 and par == 0),
                        stop=(mp == MPAIR - 1 and par == 1),
                        tile_position=(0, h * 64),
                    )

        def emit_rs(u, mp, rs_t):
            # 4x col-tiled M=1 ones-matmuls: rowsums for (h, q-quarter) at
            # psum partitions 0/32/64/96, accumulated over mp AND par so no
            # cross-partition combine is needed afterwards. F=256 each; the
            # four tiles stream concurrently (256 cyc per (mp, par) slot).
            for par in range(2):
                for quad in range(4):
                    pp = 32 * quad
                    nc.tensor.matmul(
                        rs_t[pp : pp + 1, 0:256],
                        ones_sb[:, 0:1],
                        pt[:, mp, par, quad * 256 : (quad + 1) * 256],
                        start=(mp == 0 and par == 0),
                        stop=(mp == MPAIR - 1 and par == 1),
                        tile_position=(0, pp),
                    )

        # trailing work for a finished unit u: copy O, rowsums -> rinv
        def emit_post(u, ot_t, rs_t, fast):
            p, qc = u // NQC, u % NQC
            qs = slice(qc * 512, (qc + 1) * 512)
            nc.scalar.copy(out=OT[p][:, qs], in_=ot_t)
            if fast:
                # rowsums straight into rs2 rows (h*64): quarters live at
                # psum partitions 32*(2h+g)
                off = 512 * (u % 4)
                for h in range(2):
                    for g in range(2):
                        src = rs_t[32 * (2 * h + g) : 32 * (2 * h + g) + 1, 0:256]
                        nc.vector.tensor_copy(
                            out=rs2[
                                h * 64 : h * 64 + 1,
                                off + g * 256 : off + (g + 1) * 256,
                            ],
                            in_=src,
                        )
                return
            for h in range(2):
                rs_row = small.tile(
                    [1, 512], f32, tag=f"rsrow{h}", name=f"rsr{u}_{h}"
                )
                for g in range(2):
                    nc.vector.tensor_copy(
                        out=rs_row[:, g * 256 : (g + 1) * 256],
                        in_=rs_t[32 * (2 * h + g) : 32 * (2 * h + g) + 1, 0:256],
                    )
                nc.sync.dma_start(
                    out=rs_dram[2 * p + h : 2 * p + h + 1, qs], in_=rs_row
                )
            rs_g = small.tile([128, 2, 4], f32, tag="rs", name=f"rs{u}")
            nc.sync.dma_start(
                out=rs_g,
                in_=rs_dram[2 * p : 2 * p + 2, qs].rearrange(
                    "h (p f) -> p h f", f=4
                ),
            )
            rinv_t = small.tile([128, 2, 4], f32, tag="ri", name=f"ri{u}")
            nc.vector.reciprocal(out=rinv_t, in_=rs_g)
            nc.sync.dma_start(
                out=rinv_dram[2 * p : 2 * p + 2, qs].rearrange(
                    "h (p f) -> p h f", f=4
                ),
                in_=rinv_t,
            )

        def emit_norm(u):
            p, qc = u // NQC, u % NQC
            qs = slice(qc * 512, (qc + 1) * 512)
            rbc = small.tile([128, 512], f32, tag="rbc", name=f"rbc{u}")
            for h in range(2):
                nc.sync.dma_start(
                    out=rbc[h * 64 : (h + 1) * 64, :],
                    in_=rinv_dram[
                        2 * p + h : 2 * p + h + 1, qs
                    ].to_broadcast([64, 512]),
                )
            nc.gpsimd.tensor_mul(OT[p][:, qs], OT[p][:, qs], rbc)

        def emit_norm_fast(u):
            # reciprocal off the PSUM rowsums already in rs2; partition-
            # broadcast via K=1 matmuls; multiply on DVE.
            p, qc = u // NQC, u % NQC
            qs = slice(qc * 512, (qc + 1) * 512)
            off = 512 * (u % 4)
            nc.vector.reciprocal(
                out=rinv2[:, off : off + 512], in_=rs2[:, off : off + 512]
            )
            rbc = ps_pp.tile([128, 512], f32, tag="pp", name=f"rbcf{u}")
            for h in range(2):
                nc.tensor.matmul(
                    rbc[h * 64 : (h + 1) * 64, :],
                    onesf_sb[h * 64 : h * 64 + 1, :],
                    rinv2[h * 64 : h * 64 + 1, off : off + 512],
                    start=True, stop=True,
                )
            nc.vector.tensor_mul(OT[p][:, qs], OT[p][:, qs], rbc)

        def emit_y(u, t):
            p, qc = u // NQC, u % NQC
            qt = qc * 4 + t
            ys = ys_pool.tile([128, C], bf16, tag="ys", name=f"ys{u}_{t}")
            for ch in range(2):
                ps = ps_pp.tile([128, 512], f32, tag="pp", name=f"psy{u}{t}{ch}")
                for half in range(2):
                    nc.tensor.matmul(
                        ps[half * 64 : (half + 1) * 64, :],
                        OT[p][:, qt * 128 + half * 64 : qt * 128 + (half + 1) * 64],
                        Wout_t[:, p, ch * 512 : (ch + 1) * 512],
                        start=True, stop=True,
                        tile_position=(0, half * 64),
                    )
                if ch == 0:
                    nc.scalar.copy(out=ys[:, 0:512], in_=ps)
                else:
                    nc.vector.tensor_copy(out=ys[:, 512:1024], in_=ps)
            nc.sync.dma_start(
                out=yp.ap()[p, qt * 128 : (qt + 1) * 128, :], in_=ys
            )

        # ---- weave schedules ----
        proj_sched = {
            0: [lambda: emit_q(0, 1), lambda: emit_k(0, 1),
                lambda: emit_k(0, 2), lambda: emit_k(0, 3)]
               + [lambda mt=mt: emit_v(mt) for mt in range(MT)],
            1: [lambda: emit_q(0, 2), lambda: emit_k(1, 0), lambda: emit_k(1, 1)],
            2: [lambda: emit_q(0, 3), lambda: emit_k(1, 2), lambda: emit_q(1, 0)],
            3: [lambda: emit_k(1, 3), lambda: emit_q(1, 1)],
            4: [lambda: emit_q(1, 2), lambda: emit_k(2, 0)],
            5: [lambda: emit_q(1, 3), lambda: emit_k(2, 1)],
            6: [lambda: emit_k(2, 2), lambda: emit_q(2, 0)],
            7: [lambda: emit_k(2, 3), lambda: emit_q(2, 1)],
            8: [lambda: emit_q(2, 2), lambda: emit_k(3, 0)],
            9: [lambda: emit_q(2, 3), lambda: emit_k(3, 1)],
            10: [lambda: emit_k(3, 2), lambda: emit_q(3, 0)],
            11: [lambda: emit_k(3, 3), lambda: emit_q(3, 1)],
            12: [lambda: emit_q(3, 2)],
            13: [lambda: emit_q(3, 3)],
        }

        # ---- bootstrap ----
        emit_q(0, 0)
        emit_k(0, 0)

        prev = None          # (u-1, ot_t, rs_t) for post
        norm_u = None        # unit whose norm should be emitted
        y_u = None           # unit whose Y should be emitted

        for u in range(NU):
            queue = []
            if prev is not None:
                up, pot, prs = prev
                emit_post(up, pot, prs, fast=(up >= NU - 3))
            queue.extend(proj_sched.get(u, []))
            if norm_u is not None and norm_u < NU - 3:
                queue.append(lambda nu=norm_u: emit_norm(nu))
            if y_u is not None and y_u <= NU - 4:
                for t in range(4):
                    queue.append(lambda yu=y_u, t=t: emit_y(yu, t))
            if prev is not None and prev[0] >= NU - 3:
                # fast norm is the LAST pop (mt=15): its DVE inputs (rs2
                # from the hoisted post) are long since ready, so the PE
                # broadcast matmuls never stall the in-order queue
                queue.append(lambda nu=prev[0]: emit_norm_fast(nu))

            ot_t = ps_ot.tile([128, 512], f32, tag="ot", name=f"ot{u}")
            nq = len(queue)
            popped = 0
            if u == 0:
                # AV(mp) legal only after emit_v(2mp+1) popped; queue has
                # 4 q/k emits then 16 V emits: V(j) is queue item 4+j.
                pending_av = list(range(MPAIR))
            for mt in range(MT):
                st = emit_s(u, mt)
                emit_exp(u, mt, st)
                if mt % 2 == 0:
                    continue
                # pop weave items at mt-pair boundaries so the PE array
                # mode only toggles S-block <-> (128,64)-block per pair.
                # Always pop >=1 at mt=1 so post(u-1) lands before this
                # unit's first AV reuses the single ot PSUM bank.
                want = max((mt + 1) * nq // MT, 1)
                while popped < want:
                    queue[popped]()
                    popped += 1
                if u == 0:
                    while pending_av and 4 + 2 * pending_av[0] + 1 < popped:
                        emit_av(u, pending_av.pop(0), ot_t)
                elif mt >= 5:
                    emit_av(u, (mt - 5) // 2, ot_t)
            while popped < nq:
                queue[popped]()
                popped += 1
            if u == 0:
                while pending_av:
                    emit_av(u, pending_av.pop(0), ot_t)
            # rowsum accumulator borrows a pp bank only for the unit-end
            # rs block (the bank was idle there anyway); readers in
            # post(u) release it early in unit u+1. rs(0..5) runs between
            # the last two AV slots to cover the exp(15) latency.
            rs_t = ps_pp.tile([128, 512], f32, tag="pp", name=f"rsp{u}")
            if u == 0:
                for mp in range(MPAIR):
                    emit_rs(u, mp, rs_t)
            else:
                emit_av(u, MPAIR - 2, ot_t)
                for mp in range(MPAIR - 2):
                    emit_rs(u, mp, rs_t)
                emit_av(u, MPAIR - 1, ot_t)
                emit_rs(u, MPAIR - 2, rs_t)
                emit_rs(u, MPAIR - 1, rs_t)

            prev = (u, ot_t, rs_t)
            norm_u = u - 1 if u >= 1 else None
            y_u = u - 2 if u >= 2 else None

        # ---- tail: units 13-15 were fast-normed right after their posts;
        # only unit 15's post/norm plus the deferred Ys remain ----
        up, pot, prs = prev
        emit_post(up, pot, prs, fast=True)
        for t in range(4):
            emit_y(NU - 3, t)
        emit_norm_fast(NU - 1)
        for t in range(4):
            emit_y(NU - 2, t)
        for t in range(4):
            emit_y(NU - 1, t)

    _split_excess_waits(nc)
    return nc


def make_in_maps(x, Wq, Wkv, Wout, bout):
    x = np.asarray(x, dtype=np.float32)
    Wq = np.asarray(Wq, dtype=np.float32)
    Wkv = np.asarray(Wkv, dtype=np.float32)
    Wout = np.asarray(Wout, dtype=np.float32)
    Wq_s = Wq * SCALE
    Wk = Wkv[:, :C]
    Wv = Wkv[:, C:]
    in_maps = []
    for core in range(NCORES):
        b, g = core // 2, core % 2
        cs = slice(g * 512, (g + 1) * 512)
        in_maps.append(
            dict(
                xT=np.ascontiguousarray(x[b].T).astype(_BF16),
                Wq=np.ascontiguousarray(Wq_s[:, cs]).astype(_BF16),
                Wk=np.ascontiguousarray(Wk[:, cs]).astype(_BF16),
                Wv=np.ascontiguousarray(Wv[:, cs]).astype(_BF16),
                Wout=np.ascontiguousarray(Wout[cs, :]).astype(_BF16),
            )
        )
    return in_maps


def assemble(results, bout):
    bout = np.asarray(bout, dtype=np.float32)
    out = np.empty((B, N, C), dtype=np.float32)
    for b in range(B):
        acc = results[2 * b]["yp"].astype(np.float32).sum(axis=0)
        acc += results[2 * b + 1]["yp"].astype(np.float32).sum(axis=0)
        out[b] = acc + bout
    return out


def kernel(x, Wq, Wkv, Wout, bout):
    from concourse.bass_utils import run_bass_kernel_spmd

    if "nc" not in _cache:
        _cache["nc"] = build_nc()
    in_maps = make_in_maps(x, Wq, Wkv, Wout, bout)
    res = run_bass_kernel_spmd(_cache["nc"], in_maps, core_ids=list(range(NCORES)))
    return assemble(res.results, bout)
